# revision 11
# baseline (speedup 1.0000x reference)
"""GAT message-passing kernel for TRN2: host preprocessing + Bass/Tile program.

Design (per core, SPMD over 8 cores, nodes sharded by destination block):
  phase 0: feat = x @ W_gat for own node shard (bf16), er = feat . attn_r;
           AllGather bf16 feat table [n_pad, hid] to every core's DRAM.
  edge phase (per dst block of 128 nodes, edges pre-sorted by (dst blk, src)):
    - dma_gather feat rows of the block's edge sources (bf16, 256B rows) in
      two calls per half-table (int16 gather indices cover 32k rows each).
    - one-hot masks OH[e, d] = (iota == dst_e) for the whole block in ONE
      batched DVE op (broadcast APs), stored next to the gathered G rows.
    - z = el + er in two batched DVE ops: products [G|OH] * [attn_l|er_bcast]
      then a segmented reduce; leaky on DVE; exp on ACT (table stays pinned).
    - Oa = OH * alpha (one batched DVE op).
    - aggregation TRANSPOSED on PE: pAT[h,d] += G_j^T-style matmul with the
      gathered chunk as the stationary; denominator via a ones-column
      stationary streaming the same Oa chunks.
    - tail: rec = 1/max(den,eps) broadcast via rank-1 PE matmul; gene =
      pAT*rec; +bias; leaky; out = gene @ W_lin^T directly ([h,d] layout
      needs no transpose); per-block outputs staged to an SBUF strip and
      written with one DMA.
Softmax max-subtraction is dropped (exp args bounded ~ +-8; ratios identical).
"""

import numpy as np
import ml_dtypes
from contextlib import ExitStack

import concourse.bass as bass
import concourse.tile as tile
from concourse import bacc, mybir
from concourse import library_config

dt = mybir.dt
P = 128
PAD_DST = 512.0  # one-hot miss sentinel (exact in bf16, > 127)


# ---------------------------------------------------------------- host side

def preprocess(src, dst, n_nodes, n_cores):
    """Pure index-space preprocessing (no float math on values)."""
    src = np.asarray(src).astype(np.int64)
    dst = np.asarray(dst).astype(np.int64)
    npc = n_nodes // n_cores                      # nodes per core
    assert npc * n_cores == n_nodes
    blocks = (npc + P - 1) // P
    npc_pad = blocks * P                          # padded nodes per core
    n_pad = npc_pad * n_cores                     # padded global node count
    half = n_pad // 2                             # low table rows [0, half)
    assert half <= 32767 and (n_pad - half) <= 32767
    assert half % npc_pad == 0                    # half boundary between cores

    core_of = dst // npc
    blk_of = (dst % npc) // P
    dloc_of = (dst % npc) % P
    src = (src // npc) * npc_pad + (src % npc)    # padded source coordinates

    lo_lists = [[[] for _ in range(blocks)] for _ in range(n_cores)]
    hi_lists = [[[] for _ in range(blocks)] for _ in range(n_cores)]
    order = np.lexsort((src, blk_of, core_of))
    s_s, c_s, b_s, d_s = src[order], core_of[order], blk_of[order], dloc_of[order]
    hi_mask = s_s >= half
    for c in range(n_cores):
        cm = c_s == c
        for b in range(blocks):
            m = cm & (b_s == b)
            ml = m & ~hi_mask
            mh = m & hi_mask
            lo_lists[c][b] = (s_s[ml], d_s[ml])
            hi_lists[c][b] = (s_s[mh] - half, d_s[mh])

    def nchunks(n):
        return (n + P - 1) // P

    C_lo = [max(max(nchunks(len(lo_lists[c][b][0])) for c in range(n_cores)), 1)
            for b in range(blocks)]
    C_hi = [max(nchunks(len(hi_lists[c][b][0])) for c in range(n_cores))
            for b in range(blocks)]

    total_chunks = sum(C_lo) + sum(C_hi)
    total_L = total_chunks * P

    per_core = []
    for c in range(n_cores):
        idx = np.zeros(total_L, dtype=np.int16)
        dstf = np.full(total_L, PAD_DST, dtype=np.float32)
        off = 0
        for b in range(blocks):
            for lists, C in ((lo_lists, C_lo[b]), (hi_lists, C_hi[b])):
                L = C * P
                if L == 0:
                    continue
                s_arr, d_arr = lists[c][b]
                n = len(s_arr)
                idx[off:off + n] = s_arr.astype(np.int16)
                dstf[off:off + n] = d_arr.astype(np.float32)
                off += L
        assert off == total_L
        idx16 = np.tile(idx.reshape(total_L // 16, 16).T, (8, 1)).copy()
        dstf2 = dstf.reshape(total_chunks, P).T.copy()
        per_core.append({"idx16": idx16, "dstf": dstf2})

    sched = {
        "n_nodes": n_nodes, "n_cores": n_cores, "npc": npc, "blocks": blocks,
        "npc_pad": npc_pad, "n_pad": n_pad,
        "half": half, "C_lo": C_lo, "C_hi": C_hi,
        "total_chunks": total_chunks, "total_L": total_L,
    }
    return sched, per_core


def make_core_inputs(sched, per_core, x, W_gat, attn_l, attn_r, bias_gat, W_lin):
    n_cores, npc, blocks = sched["n_cores"], sched["npc"], sched["blocks"]
    in_f = x.shape[1]
    hid = W_gat.shape[1]
    x = np.asarray(x, dtype=np.float32)
    in_maps = []
    for c in range(n_cores):
        xs = x[c * npc:(c + 1) * npc]
        xpad = np.zeros((blocks * P, in_f), dtype=np.float32)
        xpad[:npc] = xs
        m = {
            "xT": np.ascontiguousarray(xpad.T).astype(ml_dtypes.bfloat16),
            "Wg": np.asarray(W_gat, dtype=np.float32),
            "attnl_v": np.asarray(attn_l, np.float32).astype(ml_dtypes.bfloat16),
            "attnr_b": np.broadcast_to(np.asarray(attn_r, np.float32), (P, hid)).copy(),
            "bias_col": np.asarray(bias_gat, np.float32)[:, None].copy(),
            "WlT": np.ascontiguousarray(np.asarray(W_lin, np.float32).T),
            "iota_bf": np.broadcast_to(
                np.arange(P, dtype=ml_dtypes.bfloat16), (P, P)).copy(),
            "idx16": per_core[c]["idx16"],
            "dstbf": per_core[c]["dstf"].astype(ml_dtypes.bfloat16),
        }
        in_maps.append(m)
    return in_maps


# ---------------------------------------------------------------- device side

def build_program(sched, in_f, hid, out_f, attn_slope=0.2, act_slope=0.01,
                  n_repeat=1, scratch=32768, gmax=6, unroll=False):
    n_cores = sched["n_cores"]
    npc, blocks, half = sched["npc"], sched["blocks"], sched["half"]
    C_lo, C_hi = sched["C_lo"], sched["C_hi"]
    total_chunks, total_L = sched["total_chunks"], sched["total_L"]
    assert in_f % P == 0 and hid == P
    KT = in_f // P

    nc = bacc.Bacc("TRN2", target_bir_lowering=False, debug=False,
                   num_devices=n_cores, dynamic_dma_scratch_size=scratch)

    def din(name, shape, dtype):
        return nc.dram_tensor(name, shape, dtype, kind="ExternalInput").ap()

    xT = din("xT", [in_f, blocks * P], dt.bfloat16)
    Wg = din("Wg", [in_f, hid], dt.float32)
    attnl_v = din("attnl_v", [hid], dt.bfloat16)
    attnr_b = din("attnr_b", [P, hid], dt.float32)
    bias_col = din("bias_col", [P, 1], dt.float32)
    WlT = din("WlT", [hid, out_f], dt.float32)
    iota_bf = din("iota_bf", [P, P], dt.bfloat16)
    idx16 = din("idx16", [128, total_L // 16], dt.int16)
    dstbf = din("dstbf", [P, total_chunks], dt.bfloat16)
    out = nc.dram_tensor("out", [blocks * P, out_f], dt.float32,
                         kind="ExternalOutput").ap()

    tableShard = nc.dram_tensor("tableShard", [blocks * P, hid],
                                dt.bfloat16).ap()
    tableFull = nc.dram_tensor("tableFull", [sched["n_pad"], hid],
                               dt.bfloat16, addr_space="Shared").ap()
    er_lin = nc.dram_tensor("er_lin", [blocks * P], dt.bfloat16).ap()

    with ExitStack() as ctx:
        tc = ctx.enter_context(tile.TileContext(nc))
        nc.gpsimd.load_library(library_config.mlp)
        const = ctx.enter_context(tc.tile_pool(name="const", bufs=1))

        iota_bf_sb = const.tile([P, P], dt.bfloat16)
        nc.sync.dma_start(iota_bf_sb[:], iota_bf[:])
        attnr_sb = const.tile([P, hid], dt.float32)
        nc.sync.dma_start(attnr_sb[:], attnr_b[:])
        bias_sb = const.tile([P, 1], dt.float32)
        nc.sync.dma_start(bias_sb[:], bias_col[:])
        WlT_sb = const.tile([hid, out_f], dt.float32)
        nc.sync.dma_start(WlT_sb[:], WlT[:])
        ones_col_bf = const.tile([P, 1], dt.bfloat16)
        nc.vector.memset(ones_col_bf[:], 1.0)
        ones_row_f = const.tile([1, P], dt.float32)
        nc.vector.memset(ones_row_f[:], 1.0)
        idx_sb = const.tile([128, total_L // 16], dt.int16)
        nc.sync.dma_start(idx_sb[:], idx16[:])
        dst_sb = const.tile([P, total_chunks], dt.bfloat16)
        nc.sync.dma_start(dst_sb[:], dstbf[:])

        # ---- phase 0: feat shard + er + table AllGather
        ph = ctx.enter_context(tc.tile_pool(name="ph", bufs=1))
        xT_sb = []
        Wg_sb = []
        for k in range(KT):
            t = ph.tile([P, blocks * P], dt.bfloat16, tag=f"xT{k}")
            nc.sync.dma_start(t[:], xT[k * P:(k + 1) * P, :])
            xT_sb.append(t)
            wf = ph.tile([P, hid], dt.float32, tag="Wgf")
            nc.sync.dma_start(wf[:], Wg[k * P:(k + 1) * P, :])
            w = ph.tile([P, hid], dt.bfloat16, tag=f"Wg{k}")
            nc.vector.tensor_copy(w[:], wf[:])
            Wg_sb.append(w)
        er_sb = const.tile([P, blocks], dt.float32)
        er_bf = const.tile([P, blocks], dt.bfloat16)

        psA = ctx.enter_context(tc.tile_pool(name="psA", bufs=2, space="PSUM"))
        psB = ctx.enter_context(tc.tile_pool(name="psB", bufs=2, space="PSUM"))
        psR = ctx.enter_context(tc.tile_pool(name="psR", bufs=2, space="PSUM"))
        psO = ctx.enter_context(tc.tile_pool(name="psO", bufs=2, space="PSUM"))

        featp = ctx.enter_context(tc.tile_pool(name="featp", bufs=3))
        scrp = ctx.enter_context(tc.tile_pool(name="scrp", bufs=3))

        for nb in range(blocks):
            fp = psA.tile([P, hid], dt.float32, tag="psA")
            for k in range(KT):
                nc.tensor.matmul(fp[:], lhsT=xT_sb[k][:, nb * P:(nb + 1) * P],
                                 rhs=Wg_sb[k][:], start=(k == 0),
                                 stop=(k == KT - 1))
            fbf = featp.tile([P, hid], dt.bfloat16, tag="feat")
            nc.vector.tensor_copy(fbf[:], fp[:])
            scr = scrp.tile([P, hid], dt.float32, tag="scr")
            nc.vector.scalar_tensor_tensor(
                out=scr[:], in0=fp[:], scalar=1.0, in1=attnr_sb[:],
                op0=mybir.AluOpType.bypass, op1=mybir.AluOpType.mult,
                accum_out=er_sb[:, nb:nb + 1])
            nc.sync.dma_start(tableShard[nb * P:(nb + 1) * P, :], fbf[:])
        nc.vector.tensor_copy(er_bf[:], er_sb[:])
        for nb in range(blocks):
            nc.sync.dma_start(er_lin[nb * P:(nb + 1) * P, None],
                              er_bf[:, nb:nb + 1])

        nc.gpsimd.collective_compute(
            "AllGather", mybir.AluOpType.bypass,
            replica_groups=[list(range(n_cores))],
            ins=[tableShard[:].opt()], outs=[tableFull[:].opt()])

        # ---- edge phase
        gp = ctx.enter_context(tc.tile_pool(name="gp", bufs=3))      # [G | OH]
        oap = ctx.enter_context(tc.tile_pool(name="oap", bufs=3))    # Oa_all
        aep = ctx.enter_context(tc.tile_pool(name="aep", bufs=1))    # AE strip
        prp = ctx.enter_context(tc.tile_pool(name="prp", bufs=3))    # products
        sp = ctx.enter_context(tc.tile_pool(name="sp", bufs=3))      # z/alpha
        tp = ctx.enter_context(tc.tile_pool(name="tp", bufs=3))      # tail

        n_loop = 1 if unroll else n_repeat
        n_unroll = n_repeat if unroll else 1
        loop_ctx = tc.For_i(0, n_loop, 1) if n_loop > 1 else None
        if loop_ctx is not None:
            loop_ctx.__enter__()
        for _rep in range(n_unroll):
          # AE strip: [attnl | er_b] per block, interleaved [P, blocks*256]
          AE = aep.tile([P, blocks * 2 * P], dt.bfloat16, tag="AE")
          AEv = AE[:].rearrange("p (b s h) -> p b s h", s=2, h=P)
          nc.sync.dma_start(
              AEv[:, :, 0, :],
              attnl_v[:][None, None, :].broadcast_to((P, blocks, P)))
          nc.sync.dma_start(
              AEv[:, :, 1, :],
              er_lin[:][None, :].broadcast_to((P, blocks * P)).rearrange(
                  "p (b h) -> p b h", h=P))
          g = 0
          for b in range(blocks):
            Cl, Ch = C_lo[b], C_hi[b]
            C = Cl + Ch

            # T = [G (C*128) | OH (C*128)] bf16
            T = gp.tile([P, 2 * C * hid], dt.bfloat16, tag="T")
            T3 = T[:].rearrange("p (s h) -> p s h", h=hid)   # [P, 2C, 128]
            G3 = T3[:, 0:C, :]
            o16 = (g * P) // 16
            for cbase, ccnt, tbl in (
                    [(c0, min(gmax, Cl - c0), tableFull[0:half, :])
                     for c0 in range(0, Cl, gmax)] +
                    [(Cl + c0, min(gmax, Ch - c0),
                      tableFull[half:sched["n_pad"], :])
                     for c0 in range(0, Ch, gmax)]):
                nc.gpsimd.dma_gather(
                    G3[:, cbase:cbase + ccnt, :], tbl,
                    idx_sb[:, o16 + cbase * 8:o16 + (cbase + ccnt) * 8],
                    ccnt * P, ccnt * P, hid, elem_step=hid)

            # OH_all: T3[:, C:2C, :] = (iota == dst)
            iota_bc = iota_bf_sb[:].unsqueeze(1).broadcast_to((P, C, P))
            dst_bc = dst_sb[:, g:g + C].unsqueeze(2).broadcast_to((P, C, P))
            nc.vector.tensor_tensor(out=T3[:, C:2 * C, :], in0=iota_bc,
                                    in1=dst_bc, op=mybir.AluOpType.is_equal)

            # z: products then segmented reduce
            Pt = prp.tile([P, 2 * C * hid], dt.bfloat16, tag="Pt")
            T4 = T[:].rearrange("p (s c h) -> p s c h", s=2, h=hid)
            AE4 = AEv[:, b, :, :].unsqueeze(2).broadcast_to((P, 2, C, P))
            nc.vector.tensor_tensor(
                out=Pt[:].rearrange("p (s c h) -> p s c h", s=2, h=hid),
                in0=T4, in1=AE4, op=mybir.AluOpType.mult)
            z_all = sp.tile([P, C], dt.float32, tag="z")
            Pt_r = Pt[:].rearrange("p (s c h) -> p c s h", s=2, h=hid)
            nc.vector.tensor_reduce(z_all[:], Pt_r, mybir.AxisListType.XY,
                                    mybir.AluOpType.add)

            # alpha = exp(leaky(z)); leaky on DVE, exp on ACT (table pinned)
            lz = sp.tile([P, C], dt.float32, tag="lz")
            nc.vector.scalar_tensor_tensor(
                out=lz[:], in0=z_all[:], scalar=float(attn_slope), in1=z_all[:],
                op0=mybir.AluOpType.mult, op1=mybir.AluOpType.max)
            alpha_bf = sp.tile([P, C], dt.bfloat16, tag="abf")
            nc.scalar.activation(alpha_bf[:], lz[:],
                                 mybir.ActivationFunctionType.Exp)

            # Oa_all = OH_all * alpha
            Oa = oap.tile([P, C * hid], dt.bfloat16, tag="Oa")
            Oa3 = Oa[:].rearrange("p (c h) -> p c h", h=hid)
            a_bc = alpha_bf[:].unsqueeze(2).broadcast_to((P, C, P))
            nc.vector.tensor_tensor(out=Oa3, in0=T3[:, C:2 * C, :], in1=a_bc,
                                    op=mybir.AluOpType.mult)

            # aggregation (transposed), then denominator with one stationary
            pAT = psA.tile([P, hid], dt.float32, tag="psA")
            den = psB.tile([1, P], dt.float32, tag="psB")
            for j in range(C):
                nc.tensor.matmul(pAT[:], lhsT=T3[:, j, :], rhs=Oa3[:, j, :],
                                 start=(j == 0), stop=(j == C - 1))
            for j in range(C):
                nc.tensor.matmul(den[:], lhsT=ones_col_bf[:], rhs=Oa3[:, j, :],
                                 start=(j == 0), stop=(j == C - 1))
            g += C

            # tail
            den_s = tp.tile([1, P], dt.float32, tag="den")
            nc.vector.tensor_scalar(den_s[:], den[:], 1e-30, None,
                                    mybir.AluOpType.max)
            rec = tp.tile([1, P], dt.float32, tag="rec")
            nc.vector.reciprocal(rec[:], den_s[:])
            rec_ps = psR.tile([P, P], dt.float32, tag="rb")
            nc.tensor.matmul(rec_ps[:], lhsT=ones_row_f[:], rhs=rec[:],
                             start=True, stop=True)
            rec_sb = tp.tile([P, P], dt.float32, tag="recb")
            nc.vector.tensor_copy(rec_sb[:], rec_ps[:])
            gene = tp.tile([P, hid], dt.float32, tag="gene")
            nc.vector.tensor_tensor(out=gene[:], in0=pAT[:], in1=rec_sb[:],
                                    op=mybir.AluOpType.mult)
            geneB = tp.tile([P, hid], dt.float32, tag="geneB")
            nc.vector.tensor_scalar(geneB[:], gene[:], bias_sb[:, 0:1], None,
                                    mybir.AluOpType.add)
            geneL = tp.tile([P, hid], dt.float32, tag="geneL")
            nc.vector.scalar_tensor_tensor(
                out=geneL[:], in0=geneB[:], scalar=float(act_slope),
                in1=geneB[:], op0=mybir.AluOpType.mult, op1=mybir.AluOpType.max)
            o_ps = psO.tile([P, out_f], dt.float32, tag="ops")
            nc.tensor.matmul(o_ps[:], lhsT=geneL[:], rhs=WlT_sb[:],
                             start=True, stop=True)
            o_sb = tp.tile([P, out_f], dt.float32, tag="osb")
            nc.vector.tensor_copy(o_sb[:], o_ps[:])
            nc.sync.dma_start(out[b * P:(b + 1) * P, :], o_sb[:])
        if loop_ctx is not None:
            loop_ctx.__exit__(None, None, None)

    nc.compile()
    return nc


# ---------------------------------------------------------------- entry point

N_NODES, N_EDGES, IN_F, HID, OUT_F = 50000, 800000, 256, 128, 64
N_CORES = 8

_cache = {}


def kernel(x, src, dst, W_gat, attn_l, attn_r, bias_gat, W_lin):
    """Full-input GAT layer on 8 NeuronCores; returns [N_NODES, OUT_F] fp32."""
    from concourse.bass_utils import run_bass_kernel_spmd

    src = np.asarray(src)
    dst = np.asarray(dst)
    key = (src.tobytes(), dst.tobytes())
    ck = _cache.get("k")
    if ck is not None and ck[0] == key:
        sched, nc = ck[1], ck[2]
    else:
        sched, per_core = preprocess(src, dst, N_NODES, N_CORES)
        _cache["pc"] = per_core
        nc = build_program(sched, IN_F, HID, OUT_F)
        _cache["k"] = (key, sched, nc)
        ck = _cache["k"]
    sched = ck[1]
    per_core = _cache["pc"]
    in_maps = make_core_inputs(sched, per_core, x, W_gat, attn_l, attn_r,
                               bias_gat, W_lin)
    res = run_bass_kernel_spmd(nc, in_maps, core_ids=list(range(N_CORES)))
    out = np.concatenate(
        [res.results[c]["out"][:sched["npc"]] for c in range(N_CORES)], axis=0)
    return out.astype(np.float32)
